# revision 1
# baseline (speedup 1.0000x reference)
"""HOPE block kernel for 8 Trainium2 NeuronCores.

Sharding: 8 shards = (batch b in 0..3, sequence half in 0..1), 2048 tokens each.
The linear-attention memory scan is causal per (batch, head); cores owning the
second half of a sequence receive the first half as a "prefix" input and
rebuild the mid-sequence memory state M (= sum_t k_t v_t^T per head) before
scanning their own chunks. Even cores receive a zero prefix, keeping the SPMD
program uniform.

All matmuls run in float32r (fp32 storage, ~12-bit-mantissa PE path, bf16-class
throughput); LayerNorm/residual arithmetic stays fp32.
"""
import sys
if '/opt/trn_rl_repo' not in sys.path:
    sys.path.insert(0, '/opt/trn_rl_repo')

from contextlib import ExitStack
import numpy as np

import concourse.bass as bass
import concourse.tile as tile
from concourse import mybir
from concourse.bass_utils import run_bass_kernel_spmd
from concourse.masks import make_identity
from concourse.vector_clock import ScopedClock

f32 = mybir.dt.float32
f32r = mybir.dt.float32r
AF = mybir.ActivationFunctionType
ALU = mybir.AluOpType

DIM = 1024
HEADS = 16
HD = 64
B, S = 4, 4096
LEVELS = 3
HID = 4 * DIM
CHUNK = 128
EPS = 1e-5
P = 128

N_CORES = 8
T_OWN = S // 2      # tokens per core
T_PRE = S // 2      # prefix tokens (zeros on even cores)
BLK = 512           # phase-B token block
PRE_BLK = 512      # phase-A token block
D_T = DIM // P      # 8 feature tiles
H_T = HID // P      # 32 hidden tiles
TT_UP = 512         # FFN up-proj token tile
TT_DN = 256         # FFN down-proj token tile

MAX_WAITS = 1


def _split_multi_waits(nc, max_waits=MAX_WAITS):
    """Walrus in this toolchain encodes at most `max_waits` sem waits per
    instruction; split extra waits onto same-engine NOPs placed just before."""
    for f in nc.m.functions:
        for bb in f.blocks:
            insts = list(bb.instructions)
            if not any(
                i.sync_info and i.sync_info.on_wait and len(i.sync_info.on_wait) > max_waits
                for i in insts
            ):
                continue
            new = []
            for inst in insts:
                si = inst.sync_info
                waits = list(si.on_wait) if si and si.on_wait else []
                if len(waits) > max_waits:
                    head, rest = waits[:-max_waits], waits[-max_waits:]
                    while head:
                        chunk, head = head[:max_waits], head[max_waits:]
                        nop = mybir.InstNoOp(name=nc.get_next_instruction_name(), ins=[], outs=[])
                        nop.engine = inst.engine
                        nop.sync_info = mybir.SyncInfo(on_wait=chunk, on_update=[])
                        nc.register_instruction(nop, overwrite=True)
                        new.append(nop)
                    inst.sync_info = mybir.SyncInfo(
                        on_wait=rest, on_update=list(si.on_update) if si.on_update else [])
                new.append(inst)
            bb.instructions = new


def _layernorm_tile(nc, pools, x_t, g_bc, b_bc, eps_t, out_r):
    """LayerNorm of one [128, DIM] fp32 tile -> f32r tile (token-major)."""
    w = pools
    BNF = nc.vector.BN_STATS_FMAX
    nsub = DIM // BNF
    stats = w.tile([P, nsub, nc.vector.BN_STATS_DIM], f32, tag="ln_stats")
    xg = x_t[:].rearrange("p (s f) -> p s f", f=BNF)
    for s_ in range(nsub):
        nc.vector.bn_stats(out=stats[:, s_, :], in_=xg[:, s_, :])
    mv = w.tile([P, nc.vector.BN_AGGR_DIM], f32, tag="ln_mv")
    nc.vector.bn_aggr(out=mv, in_=stats)
    rstd = w.tile([P, 1], f32, tag="ln_rstd")
    nc.scalar.activation(out=rstd, in_=mv[:, 1:2], func=AF.Sqrt, bias=eps_t, scale=1.0)
    nc.vector.reciprocal(out=rstd, in_=rstd)
    tmp = w.tile([P, DIM], f32, tag="ln_tmp")
    nc.vector.tensor_scalar(out=tmp, in0=x_t, scalar1=mv[:, 0:1], scalar2=rstd,
                            op0=ALU.subtract, op1=ALU.mult)
    nc.vector.tensor_mul(out=tmp, in0=tmp, in1=g_bc)
    nc.vector.tensor_add(out=out_r, in0=tmp, in1=b_bc)


def _transpose_block(nc, sb_pool, ps_pool, ident, src_r, n_tok_tiles, out_tile, out_map):
    """PE-transpose token-major f32r [n_tok_tiles*128, DIM] (given per-tile via
    src_r(t) -> AP [128, DIM]) into feature-major layout via out_map(t, f) -> AP
    [128,128] destination slices."""
    for t in range(n_tok_tiles):
        src = src_r(t)
        for fidx in range(D_T):
            ps = ps_pool.tile([P, P], f32r, tag="tp_ps")
            nc.tensor.transpose(ps, src[:, fidx * P:(fidx + 1) * P], ident)
            nc.scalar.copy(out=out_map(t, fidx), in_=ps)


def build_kernel(t_own=T_OWN, t_pre=T_PRE, debug_outputs=False):
    nc = bass.Bass()

    x_own = nc.dram_tensor("x_own", [t_own, DIM], f32, kind="ExternalInput")
    x_pre = nc.dram_tensor("x_pre", [t_pre, DIM], f32, kind="ExternalInput")
    wq = nc.dram_tensor("wq", [DIM, DIM], f32r, kind="ExternalInput")
    wk = nc.dram_tensor("wk", [DIM, DIM], f32r, kind="ExternalInput")
    wv = nc.dram_tensor("wv", [DIM, DIM], f32r, kind="ExternalInput")
    wo = nc.dram_tensor("wo", [DIM, DIM], f32r, kind="ExternalInput")
    ln1_g = nc.dram_tensor("ln1_g", [DIM], f32, kind="ExternalInput")
    ln1_b = nc.dram_tensor("ln1_b", [DIM], f32, kind="ExternalInput")
    ln2_g = nc.dram_tensor("ln2_g", [DIM], f32, kind="ExternalInput")
    ln2_b = nc.dram_tensor("ln2_b", [DIM], f32, kind="ExternalInput")
    cms_w1 = nc.dram_tensor("cms_w1", [LEVELS, DIM, HID], f32r, kind="ExternalInput")
    cms_b1 = nc.dram_tensor("cms_b1", [LEVELS, HID], f32, kind="ExternalInput")
    cms_w2 = nc.dram_tensor("cms_w2", [LEVELS, HID, DIM], f32r, kind="ExternalInput")
    cms_b2 = nc.dram_tensor("cms_b2", [LEVELS, DIM], f32, kind="ExternalInput")
    maskT = nc.dram_tensor("maskT", [CHUNK, CHUNK], f32, kind="ExternalInput")
    out = nc.dram_tensor("out", [t_own, DIM], f32, kind="ExternalOutput")
    x2_dbg = None
    if debug_outputs:
        x2_dbg = nc.dram_tensor("x2_dbg", [t_own, DIM], f32, kind="ExternalOutput")

    n_own_t = t_own // P           # 128-token tiles
    n_blk = t_own // BLK           # phase-B blocks
    n_pre_blk = t_pre // PRE_BLK   # phase-A blocks
    n_tt_up = t_own // TT_UP
    n_tt_dn = t_own // TT_DN

    with tile.TileContext(nc) as tc, ExitStack() as top:
        dram = top.enter_context(tc.tile_pool(name="dram", bufs=1, space="DRAM"))
        x2_d = dram.tile([n_own_t, P, DIM], f32)
        hT_a = dram.tile([P, D_T, t_own], f32r)
        hT_b = dram.tile([P, D_T, t_own], f32r)
        upg_d = dram.tile([P, H_T, t_own], f32r)

        consts = top.enter_context(tc.tile_pool(name="consts", bufs=1))
        ident = consts.tile([P, P], f32r)
        ident_f = consts.tile([P, P], f32)
        make_identity(nc, ident_f)
        nc.vector.tensor_copy(out=ident, in_=ident_f)

        # pools live only through phases A+B (attention); freed before the FFN
        ab_stack = ExitStack()
        ab_consts = ab_stack.enter_context(tc.tile_pool(name="ab_consts", bufs=1))
        eps_t = ab_consts.tile([P, 1], f32)
        nc.vector.memset(eps_t, EPS)
        mask_t = ab_consts.tile([CHUNK, CHUNK], f32)
        nc.sync.dma_start(out=mask_t, in_=maskT.ap())
        g1 = ab_consts.tile([P, DIM], f32)
        b1 = ab_consts.tile([P, DIM], f32)
        g2 = ab_consts.tile([P, DIM], f32)
        b2 = ab_consts.tile([P, DIM], f32)
        nc.sync.dma_start(out=g1, in_=ln1_g.ap()[None, :].partition_broadcast(P).opt())
        nc.sync.dma_start(out=b1, in_=ln1_b.ap()[None, :].partition_broadcast(P).opt())
        nc.sync.dma_start(out=g2, in_=ln2_g.ap()[None, :].partition_broadcast(P).opt())
        nc.sync.dma_start(out=b2, in_=ln2_b.ap()[None, :].partition_broadcast(P).opt())

        # persistent attention memory state; head h lives at partitions
        # (h%2)*HD .. +HD so that reads share the base partition of qcT/kcT
        mt_pool = ab_stack.enter_context(tc.tile_pool(name="mt", bufs=1))
        Mt_f = [mt_pool.tile([P, HD], f32, name=f"Mt_f{h}", tag=f"Mt_f{h}") for h in range(HEADS)]
        Mt_s = [mt_pool.tile([P, HD], f32r, name=f"Mt_s{h}", tag=f"Mt_s{h}") for h in range(HEADS)]
        for h in range(HEADS):
            nc.vector.memset(Mt_f[h], 0.0)

        ln_w = ab_stack.enter_context(tc.tile_pool(name="ln_w", bufs=1))

        # ---------------- Phase A: prefix -> Mt ----------------
        for blk in range(n_pre_blk):
            with ExitStack() as ph:
                sb = ph.enter_context(tc.tile_pool(name="A_sb", bufs=1))
                ps = ph.enter_context(tc.tile_pool(name="A_ps", bufs=2, space="PSUM"))
                wstream = ph.enter_context(tc.tile_pool(name="A_w", bufs=2))
                ntt = PRE_BLK // P
                hpT = sb.tile([P, D_T, PRE_BLK], f32r)
                for t in range(ntt):
                    x_t = sb.tile([P, DIM], f32, tag=f"A_x{t % 2}")
                    nc.sync.dma_start(out=x_t, in_=x_pre.ap()[(blk * ntt + t) * P:(blk * ntt + t + 1) * P, :])
                    h_r = sb.tile([P, DIM], f32r, tag=f"A_h{t % 2}")
                    _layernorm_tile(nc, ln_w, x_t, g1, b1, eps_t, h_r)
                    for fidx in range(D_T):
                        tps = ps.tile([P, P], f32r, tag="tp_ps")
                        nc.tensor.transpose(tps, h_r[:, fidx * P:(fidx + 1) * P], ident)
                        nc.scalar.copy(out=hpT[:, fidx, t * P:(t + 1) * P], in_=tps)
                # kc/vc token-major: out[t, f] via lhsT=hpT tiles, rhs=w slices
                kc = sb.tile([P, ntt, DIM], f32r)
                vc = sb.tile([P, ntt, DIM], f32r)
                for (w_in, dst) in ((wk, kc), (wv, vc)):
                    w_all = w_in.ap().rearrange("(kt p) d -> p kt d", p=P)
                    for nh in range(2):
                        w_t = wstream.tile([P, D_T, 512], f32r, tag="A_wt")
                        nc.sync.dma_start(out=w_t, in_=w_all[:, :, nh * 512:(nh + 1) * 512])
                        for m in range(ntt):
                            pst = ps.tile([P, 512], f32, tag="A_pst")
                            for k in range(D_T):
                                nc.tensor.matmul(pst, hpT[:, k, m * P:(m + 1) * P], w_t[:, k, :],
                                                 start=(k == 0), stop=(k == D_T - 1))
                            nc.scalar.copy(out=dst[:, m, nh * 512:(nh + 1) * 512], in_=pst)
                # accumulate Mt += kc^T vc per head, chunks of 128 tokens
                for h in range(HEADS):
                    pb = (h % 2) * HD
                    mt_ps = ps.tile([HD, HD], f32, tag="A_mt")
                    for ch in range(ntt):
                        nc.tensor.matmul(mt_ps, kc[:, ch, h * HD:(h + 1) * HD],
                                         vc[:, ch, h * HD:(h + 1) * HD],
                                         start=(ch == 0), stop=(ch == ntt - 1))
                    nc.vector.tensor_add(out=Mt_f[h][pb:pb + HD, :], in0=Mt_f[h][pb:pb + HD, :],
                                         in1=mt_ps)
        for h in range(HEADS):
            pb = (h % 2) * HD
            nc.scalar.copy(out=Mt_s[h][pb:pb + HD, :], in_=Mt_f[h][pb:pb + HD, :])

        # ---------------- Phase B: own tokens, attention ----------------
        wo_pool = ab_stack.enter_context(tc.tile_pool(name="wo_pool", bufs=1))
        wo_all = wo.ap().rearrange("(kt p) d -> p kt d", p=P)
        wo_ts = []
        for nh in range(2):
            w_t = wo_pool.tile([P, D_T, 512], f32r, tag=f"wo{nh}", name=f"wo_full_{nh}")
            nc.scalar.dma_start(out=w_t, in_=wo_all[:, :, nh * 512:(nh + 1) * 512])
            wo_ts.append(w_t)
        for blk in range(n_blk):
            ntt = BLK // P  # 4 token tiles per block
            tok0 = blk * BLK
            with ExitStack() as ph:
                sb = ph.enter_context(tc.tile_pool(name="B_sb", bufs=1))
                wstream = ph.enter_context(tc.tile_pool(name="B_w", bufs=2))
                with ExitStack() as sub:
                    ps = sub.enter_context(tc.tile_pool(name="B_ps", bufs=2, space="PSUM"))
                    hT_pool = sub.enter_context(tc.tile_pool(name="B_hT", bufs=1))
                    hT = hT_pool.tile([P, D_T, BLK], f32r)
                    for t in range(ntt):
                        x_t = sb.tile([P, DIM], f32, tag=f"B_x{t % 2}")
                        nc.sync.dma_start(out=x_t, in_=x_own.ap()[tok0 + t * P:tok0 + (t + 1) * P, :])
                        h_r = sb.tile([P, DIM], f32r, tag=f"B_h{t % 2}")
                        _layernorm_tile(nc, ln_w, x_t, g1, b1, eps_t, h_r)
                        for fidx in range(D_T):
                            tps = ps.tile([P, P], f32r, tag="tp_ps")
                            nc.tensor.transpose(tps, h_r[:, fidx * P:(fidx + 1) * P], ident)
                            nc.scalar.copy(out=hT[:, fidx, t * P:(t + 1) * P], in_=tps)
                    # qT, kT feature-major [128, D_T, BLK]; v token-major [128, ntt, DIM]
                    qT = sb.tile([P, D_T, BLK], f32r)
                    kT = sb.tile([P, D_T, BLK], f32r)
                    for (w_in, dst) in ((wq, qT), (wk, kT)):
                        w_all = w_in.ap().rearrange("(kt p) d -> p kt d", p=P)
                        for m in range(D_T):
                            w_t = wstream.tile([P, D_T, P], f32r, tag="B_wt")
                            nc.sync.dma_start(out=w_t, in_=w_all[:, :, m * P:(m + 1) * P])
                            pst = ps.tile([P, BLK], f32, tag="B_pst")
                            for k in range(D_T):
                                nc.tensor.matmul(pst, w_t[:, k, :], hT[:, k, :],
                                                 start=(k == 0), stop=(k == D_T - 1))
                            nc.scalar.copy(out=dst[:, m, :], in_=pst)
                    v = sb.tile([P, ntt, DIM], f32r)
                    wv_all = wv.ap().rearrange("(kt p) d -> p kt d", p=P)
                    for nh in range(2):
                        w_t = wstream.tile([P, D_T, 512], f32r, tag="B_wtv")
                        nc.sync.dma_start(out=w_t, in_=wv_all[:, :, nh * 512:(nh + 1) * 512])
                        for m in range(ntt):
                            pst = ps.tile([P, 512], f32, tag="B_pstv")
                            for k in range(D_T):
                                nc.tensor.matmul(pst, hT[:, k, m * P:(m + 1) * P], w_t[:, k, :],
                                                 start=(k == 0), stop=(k == D_T - 1))
                            nc.scalar.copy(out=v[:, m, nh * 512:(nh + 1) * 512], in_=pst)
                # scan
                y = sb.tile([P, ntt, DIM], f32r)
                with ExitStack() as sub:
                    ps_sc = sub.enter_context(tc.tile_pool(name="B_ps_sc", bufs=2, space="PSUM"))
                    ps_y = sub.enter_context(tc.tile_pool(name="B_ps_y", bufs=2, space="PSUM"))
                    ps_mt = sub.enter_context(tc.tile_pool(name="B_ps_mt", bufs=2, space="PSUM"))
                    ps_kc = sub.enter_context(tc.tile_pool(name="B_ps_kc", bufs=2, space="PSUM"))
                    scw = sub.enter_context(tc.tile_pool(name="B_scw", bufs=3))
                    for h in range(HEADS):
                        pb = (h % 2) * HD
                        fi = h // 2
                        for ch in range(ntt):
                            qcT = qT[pb:pb + HD, fi, ch * P:(ch + 1) * P]
                            kcT = kT[pb:pb + HD, fi, ch * P:(ch + 1) * P]
                            vc = v[:, ch, h * HD:(h + 1) * HD]
                            # kc = kcT^T (token-major)
                            kc_ps = ps_kc.tile([P, HD], f32r, tag="kc")
                            nc.tensor.transpose(kc_ps, kcT, ident[pb:pb + HD, pb:pb + HD])
                            kc_s = scw.tile([P, HD], f32r, tag="kc_s")
                            nc.scalar.copy(out=kc_s, in_=kc_ps)
                            # scoresT = kcT^T-contract: out[e,c] = sum_d kcT[d,e] qcT[d,c]
                            sc_ps = ps_sc.tile([P, P], f32, tag="sc")
                            nc.tensor.matmul(sc_ps, kcT, qcT, start=True, stop=True)
                            sc_r = scw.tile([P, P], f32r, tag="sc_r")
                            nc.vector.tensor_mul(out=sc_r, in0=sc_ps, in1=mask_t)
                            # y = scores^T-contract intra + inter
                            y_ps = ps_y.tile([P, HD], f32, tag="y")
                            nc.tensor.matmul(y_ps, sc_r, vc, start=True, stop=False)
                            nc.tensor.matmul(y_ps, qcT, Mt_s[h][pb:pb + HD, :], start=False, stop=True)
                            nc.scalar.copy(out=y[:, ch, h * HD:(h + 1) * HD], in_=y_ps)
                            # Mt += kc^T vc
                            mt_ps = ps_mt.tile([HD, HD], f32, tag="mt")
                            nc.tensor.matmul(mt_ps, kc_s, vc, start=True, stop=True)
                            nc.vector.tensor_add(out=Mt_f[h][pb:pb + HD, :],
                                                 in0=Mt_f[h][pb:pb + HD, :], in1=mt_ps)
                            nc.scalar.copy(out=Mt_s[h][pb:pb + HD, :], in_=Mt_f[h][pb:pb + HD, :])
                # yT, attn-out + residual, LN2, h2T
                with ExitStack() as sub:
                    ps = sub.enter_context(tc.tile_pool(name="B_ps2", bufs=2, space="PSUM"))
                    ps_at = sub.enter_context(tc.tile_pool(name="B_ps_at", bufs=2, space="PSUM"))
                    yT_pool = sub.enter_context(tc.tile_pool(name="B_yT", bufs=2))
                    x2p = sub.enter_context(tc.tile_pool(name="B_x2p", bufs=1))
                    for m in range(ntt):
                        yT_m = yT_pool.tile([P, D_T, P], f32r, tag="yT_m")
                        for fidx in range(D_T):
                            tps = ps.tile([P, P], f32r, tag="tp_ps")
                            nc.tensor.transpose(tps, y[:, m, fidx * P:(fidx + 1) * P], ident)
                            nc.scalar.copy(out=yT_m[:, fidx, :], in_=tps)
                        x_t = x2p.tile([P, DIM], f32, tag="B_x2t")
                        nc.sync.dma_start(out=x_t, in_=x_own.ap()[tok0 + m * P:tok0 + (m + 1) * P, :])
                        x2_t = x2p.tile([P, DIM], f32, tag="B_x2")
                        for nh in range(2):
                            pst = ps_at.tile([P, 512], f32, tag="B_at")
                            for k in range(D_T):
                                nc.tensor.matmul(pst, yT_m[:, k, :], wo_ts[nh][:, k, :],
                                                 start=(k == 0), stop=(k == D_T - 1))
                            nc.vector.tensor_add(out=x2_t[:, nh * 512:(nh + 1) * 512],
                                                 in0=x_t[:, nh * 512:(nh + 1) * 512], in1=pst)
                        ti = (tok0 // P) + m
                        nc.scalar.dma_start(out=x2_d[ti], in_=x2_t)
                        if debug_outputs:
                            nc.sync.dma_start(out=x2_dbg.ap()[ti * P:(ti + 1) * P, :], in_=x2_t)
                        h2_r = x2p.tile([P, DIM], f32r, tag="B_h2")
                        _layernorm_tile(nc, ln_w, x2_t, g2, b2, eps_t, h2_r)
                        for fidx in range(D_T):
                            tps = ps.tile([P, P], f32r, tag="B_h2tp")
                            nc.tensor.transpose(tps, h2_r[:, fidx * P:(fidx + 1) * P], ident)
                            h2T_s = x2p.tile([P, P], f32r, tag="B_h2T")
                            nc.scalar.copy(out=h2T_s, in_=tps)
                            nc.scalar.dma_start(out=hT_a[:, fidx, ti * P:(ti + 1) * P], in_=h2T_s)

        ab_stack.close()

        # ---------------- Phase C: CMS FFN levels ----------------
        hT_io = [(hT_a, hT_b), (hT_b, hT_a), (hT_a, hT_b)]
        TT_D = 512
        n_tt_d = t_own // TT_D
        HK = H_T // 2  # half of the hidden k-tiles
        for lvl in range(LEVELS):
            hT_in, hT_out = hT_io[lvl]
            with ExitStack() as ph:
                b1s = ph.enter_context(tc.tile_pool(name=f"C{lvl}_b", bufs=1))
                b1_t = b1s.tile([P, H_T], f32)
                nc.sync.dma_start(out=b1_t, in_=cms_b1.ap()[lvl].rearrange("(m p) -> p m", p=P))
                b2_t = b1s.tile([P, D_T], f32)
                nc.sync.dma_start(out=b2_t, in_=cms_b2.ap()[lvl].rearrange("(m p) -> p m", p=P))
                # w2 pool opened before UP so its 16 MiB load overlaps up compute
                w2sb = ph.enter_context(tc.tile_pool(name=f"C{lvl}_w2", bufs=1))
                w2_sb = w2sb.tile([P, H_T, DIM], f32r)
                w2_all = cms_w2.ap()[lvl].rearrange("(kt p) d -> p kt d", p=P)
                for kh in range(2):
                    nc.scalar.dma_start(out=w2_sb[:, kh * HK:(kh + 1) * HK, :],
                                        in_=w2_all[:, kh * HK:(kh + 1) * HK, :])
                # UP: hT_in resident, w1 streamed; upg -> DRAM
                with ExitStack() as sub:
                    sb = sub.enter_context(tc.tile_pool(name=f"C{lvl}_up_sb", bufs=1))
                    wst = sub.enter_context(tc.tile_pool(name=f"C{lvl}_up_w", bufs=2))
                    ps = sub.enter_context(tc.tile_pool(name=f"C{lvl}_up_ps", bufs=8, space="PSUM"))
                    ostg = sub.enter_context(tc.tile_pool(name=f"C{lvl}_up_o", bufs=3))
                    hT_sb = sb.tile([P, D_T, t_own], f32r)
                    for k in range(D_T):
                        for tc_ in range(n_tt_up):
                            nc.sync.dma_start(out=hT_sb[:, k, tc_ * TT_UP:(tc_ + 1) * TT_UP],
                                              in_=hT_in[:, k, tc_ * TT_UP:(tc_ + 1) * TT_UP])
                    w1_all = cms_w1.ap()[lvl].rearrange("(kt p) d -> p kt d", p=P)
                    for m in range(H_T):
                        w_t = wst.tile([P, D_T, P], f32r, tag="up_w")
                        nc.sync.dma_start(out=w_t, in_=w1_all[:, :, m * P:(m + 1) * P])
                        psl = [ps.tile([P, TT_UP], f32, tag="up_ps", name=f"up_ps_{m}_{i}") for i in range(n_tt_up)]
                        for k in range(D_T):
                            for tt in range(n_tt_up):
                                nc.tensor.matmul(psl[tt], w_t[:, k, :], hT_sb[:, k, tt * TT_UP:(tt + 1) * TT_UP],
                                                 start=(k == 0), stop=(k == D_T - 1))
                        for tt in range(n_tt_up):
                            og = ostg.tile([P, TT_UP], f32r, tag="up_og")
                            nc.scalar.activation(out=og, in_=psl[tt], func=AF.Gelu_apprx_tanh,
                                                 bias=b1_t[:, m:m + 1], scale=1.0)
                            nc.sync.dma_start(out=upg_d[:, m, tt * TT_UP:(tt + 1) * TT_UP], in_=og)
                # DOWN: w2 resident; upg streamed in half-K slices (ping-pong)
                with ExitStack() as sub:
                    sb = sub.enter_context(tc.tile_pool(name=f"C{lvl}_dn_sb", bufs=2))
                    ps = sub.enter_context(tc.tile_pool(name=f"C{lvl}_dn_ps", bufs=8, space="PSUM"))
                    ostg = sub.enter_context(tc.tile_pool(name=f"C{lvl}_dn_o", bufs=3))
                    for tt in range(n_tt_d):
                        ug_h = []
                        for kh in range(2):
                            ug = sb.tile([P, HK, TT_D], f32r, tag="dn_ug", name=f"dn_ug_{lvl}_{tt}_{kh}")
                            nc.sync.dma_start(
                                out=ug, in_=upg_d[:, kh * HK:(kh + 1) * HK, tt * TT_D:(tt + 1) * TT_D])
                            ug_h.append(ug)
                        pstl = [ps.tile([P, TT_D], f32, tag="dn_ps", name=f"dn_ps_{lvl}_{tt}_{m}")
                                for m in range(D_T)]
                        for kh in range(2):
                            for m in range(D_T):
                                for k in range(HK):
                                    nc.tensor.matmul(pstl[m], w2_sb[:, kh * HK + k, m * P:(m + 1) * P],
                                                     ug_h[kh][:, k, :],
                                                     start=(kh == 0 and k == 0),
                                                     stop=(kh == 1 and k == HK - 1))
                        for m in range(D_T):
                            og = ostg.tile([P, TT_D], f32r, tag="dn_og")
                            nc.vector.tensor_scalar_add(out=og, in0=pstl[m], scalar1=b2_t[:, m:m + 1])
                            nc.sync.dma_start(out=hT_out[:, m, tt * TT_D:(tt + 1) * TT_D], in_=og)

        # ---------------- Phase D: out = x2 + h^T ----------------
        hT_fin = hT_io[LEVELS - 1][1]
        with ExitStack() as ph:
            sb = ph.enter_context(tc.tile_pool(name="D_sb", bufs=3))
            ps = ph.enter_context(tc.tile_pool(name="D_ps", bufs=3, space="PSUM"))
            for t in range(n_own_t):
                x2_t = sb.tile([P, DIM], f32, tag="D_x2")
                nc.sync.dma_start(out=x2_t, in_=x2_d[t])
                o_t = sb.tile([P, DIM], f32, tag="D_o")
                hsl = sb.tile([P, D_T, P], f32r, tag="D_h")
                nc.sync.dma_start(out=hsl, in_=hT_fin[:, :, t * P:(t + 1) * P])
                for fidx in range(D_T):
                    tps = ps.tile([P, P], f32r, tag="D_tp")
                    nc.tensor.transpose(tps, hsl[:, fidx, :], ident)
                    nc.vector.tensor_add(out=o_t[:, fidx * P:(fidx + 1) * P],
                                         in0=x2_t[:, fidx * P:(fidx + 1) * P],
                                         in1=tps.bitcast(f32))
                nc.scalar.dma_start(out=out.ap()[t * P:(t + 1) * P, :], in_=o_t)

    _split_multi_waits(nc)
    return nc


_NC_CACHE = {}
LAST_RESULT = None


def _get_nc(key, **kw):
    if key not in _NC_CACHE:
        _NC_CACHE[key] = build_kernel(**kw)
    return _NC_CACHE[key]


def kernel(x, ln1_g, ln1_b, wq, wk, wv, wo, ln2_g, ln2_b,
           cms_w1, cms_b1, cms_w2, cms_b2, **extra):
    x = np.asarray(x, np.float32)
    maskT = np.triu(np.ones((CHUNK, CHUNK), np.float32))  # maskT[e,c] = e<=c
    common = {
        "wq": np.asarray(wq, np.float32), "wk": np.asarray(wk, np.float32),
        "wv": np.asarray(wv, np.float32), "wo": np.asarray(wo, np.float32),
        "ln1_g": np.asarray(ln1_g, np.float32), "ln1_b": np.asarray(ln1_b, np.float32),
        "ln2_g": np.asarray(ln2_g, np.float32), "ln2_b": np.asarray(ln2_b, np.float32),
        "cms_w1": np.asarray(cms_w1, np.float32), "cms_b1": np.asarray(cms_b1, np.float32),
        "cms_w2": np.asarray(cms_w2, np.float32), "cms_b2": np.asarray(cms_b2, np.float32),
        "maskT": maskT,
    }
    zeros_pre = np.zeros((T_PRE, DIM), np.float32)
    in_maps = []
    for c in range(N_CORES):
        b, half = c // 2, c % 2
        own = x[b, half * T_OWN:(half + 1) * T_OWN]
        pre = x[b, 0:T_PRE] if half else zeros_pre
        in_maps.append({**common, "x_own": np.ascontiguousarray(own),
                        "x_pre": np.ascontiguousarray(pre)})
    nc = _get_nc("full")
    res = run_bass_kernel_spmd(nc, in_maps, core_ids=list(range(N_CORES)))
    global LAST_RESULT
    LAST_RESULT = res
    out = np.empty((B, S, DIM), np.float32)
    for c in range(N_CORES):
        b, half = c // 2, c % 2
        out[b, half * T_OWN:(half + 1) * T_OWN] = res.results[c]["out"]
    return out



# revision 7
# speedup vs baseline: 1.9476x; 1.9476x over previous
"""HOPE block kernel for 8 Trainium2 NeuronCores.

Sharding: 8 shards = (batch b in 0..3, sequence half in 0..1), 2048 tokens each.
Odd cores rebuild the mid-sequence linear-attention memory M from the first
half ("prefix"); even cores get a zero prefix (uniform SPMD program).

Precision plan (validated vs fp32 reference, rel err ~5e-4):
 - Attention path in fp16 (projections, scan, M-state reads); accumulation and
   the M accumulator itself stay fp32.
 - CMS FFN in fp8(e4m3, clip 240) with DoubleRow matmuls (2 k-tiles per
   instruction); weights host-prescaled by 64, descaled at PSUM eviction.
 - The hidden intermediate ug (tokens x 4096) lives entirely in SBUF as fp8 -
   no DRAM round trip.
 - Level-2 down-projection emits token-major output and fuses the residual
   add, so no final transpose pass.
"""
import sys
if '/opt/trn_rl_repo' not in sys.path:
    sys.path.insert(0, '/opt/trn_rl_repo')

from contextlib import ExitStack
import numpy as np
import ml_dtypes

import concourse.bass as bass
import concourse.tile as tile
from concourse import mybir
from concourse.bass_utils import run_bass_kernel_spmd
from concourse.masks import make_identity

f32 = mybir.dt.float32
f16 = mybir.dt.float16
f8 = mybir.dt.float8e4
AF = mybir.ActivationFunctionType
ALU = mybir.AluOpType
DR = mybir.MatmulPerfMode.DoubleRow

DIM = 1024
HEADS = 16
HD = 64
B, S = 4, 4096
LEVELS = 3
HID = 4 * DIM
CHUNK = 128
EPS = 1e-5
P = 128

N_CORES = 8
T_OWN = S // 2
T_PRE = S // 2
BLK = 512
D_T = DIM // P      # 8 feature tiles
H_T = HID // P      # 32 hidden tiles
W_SCALE = 64.0      # fp8 weight prescale (host side)

MAX_WAITS = 1


def _split_multi_waits(nc, max_waits=MAX_WAITS):
    """Walrus encodes at most `max_waits` sem waits per instruction; split
    extras onto same-engine NOPs placed just before."""
    for f in nc.m.functions:
        for bb in f.blocks:
            insts = list(bb.instructions)
            if not any(
                i.sync_info and i.sync_info.on_wait and len(i.sync_info.on_wait) > max_waits
                for i in insts
            ):
                continue
            new = []
            for inst in insts:
                si = inst.sync_info
                waits = list(si.on_wait) if si and si.on_wait else []
                if len(waits) > max_waits:
                    head, rest = waits[:-max_waits], waits[-max_waits:]
                    while head:
                        chunk, head = head[:max_waits], head[max_waits:]
                        nop = mybir.InstNoOp(name=nc.get_next_instruction_name(), ins=[], outs=[])
                        nop.engine = inst.engine
                        nop.sync_info = mybir.SyncInfo(on_wait=chunk, on_update=[])
                        nc.register_instruction(nop, overwrite=True)
                        new.append(nop)
                    inst.sync_info = mybir.SyncInfo(
                        on_wait=rest, on_update=list(si.on_update) if si.on_update else [])
                new.append(inst)
            bb.instructions = new


def _layernorm_tile(nc, w, x_t, g_bc, b_bc, eps_t, out_r):
    """LayerNorm of one [128, DIM] fp32 tile -> out_r (any dtype)."""
    BNF = nc.vector.BN_STATS_FMAX
    nsub = DIM // BNF
    stats = w.tile([P, nsub, nc.vector.BN_STATS_DIM], f32, tag="ln_stats", bufs=2)
    xg = x_t[:].rearrange("p (s f) -> p s f", f=BNF)
    for s_ in range(nsub):
        nc.vector.bn_stats(out=stats[:, s_, :], in_=xg[:, s_, :])
    mv = w.tile([P, nc.vector.BN_AGGR_DIM], f32, tag="ln_mv", bufs=2)
    nc.vector.bn_aggr(out=mv, in_=stats)
    rstd = w.tile([P, 1], f32, tag="ln_rstd", bufs=2)
    nc.scalar.activation(out=rstd, in_=mv[:, 1:2], func=AF.Sqrt, bias=eps_t, scale=1.0)
    nc.vector.reciprocal(out=rstd, in_=rstd)
    tmp = w.tile([P, DIM], f32, tag="ln_tmp", bufs=2)
    nc.vector.tensor_scalar(out=tmp, in0=x_t, scalar1=mv[:, 0:1], scalar2=rstd,
                            op0=ALU.subtract, op1=ALU.mult)
    nc.vector.tensor_mul(out=tmp, in0=tmp, in1=g_bc)
    nc.vector.tensor_add(out=out_r, in0=tmp, in1=b_bc)


def build_kernel(t_own=T_OWN, t_pre=T_PRE):
    nc = bass.Bass()

    x_own = nc.dram_tensor("x_own", [t_own, DIM], f32, kind="ExternalInput")
    x_pre = nc.dram_tensor("x_pre", [t_pre, DIM], f32, kind="ExternalInput")
    wq = nc.dram_tensor("wq", [DIM, DIM], f16, kind="ExternalInput")
    wk = nc.dram_tensor("wk", [DIM, DIM], f16, kind="ExternalInput")
    wv = nc.dram_tensor("wv", [DIM, DIM], f16, kind="ExternalInput")
    wo = nc.dram_tensor("wo", [DIM, DIM], f16, kind="ExternalInput")
    ln1_g = nc.dram_tensor("ln1_g", [DIM], f32, kind="ExternalInput")
    ln1_b = nc.dram_tensor("ln1_b", [DIM], f32, kind="ExternalInput")
    ln2_g = nc.dram_tensor("ln2_g", [DIM], f32, kind="ExternalInput")
    ln2_b = nc.dram_tensor("ln2_b", [DIM], f32, kind="ExternalInput")
    w1q = nc.dram_tensor("w1q", [LEVELS, DIM, HID], f8, kind="ExternalInput")
    cms_b1 = nc.dram_tensor("cms_b1", [LEVELS, HID], f32, kind="ExternalInput")
    w2q = nc.dram_tensor("w2q", [LEVELS, HID, DIM], f8, kind="ExternalInput")
    cms_b2 = nc.dram_tensor("cms_b2", [LEVELS, DIM], f32, kind="ExternalInput")
    maskT = nc.dram_tensor("maskT", [CHUNK, CHUNK], f32, kind="ExternalInput")
    out = nc.dram_tensor("out", [t_own, DIM], f32, kind="ExternalOutput")

    n_own_t = t_own // P
    n_blk = t_own // BLK
    n_pre_blk = t_pre // BLK

    with tile.TileContext(nc) as tc, ExitStack() as top:
        dram = top.enter_context(tc.tile_pool(name="dram", bufs=1, space="DRAM"))
        x2_d = dram.tile([n_own_t, P, DIM], f32)

        consts = top.enter_context(tc.tile_pool(name="consts", bufs=1))
        ident_f = consts.tile([P, P], f32)
        make_identity(nc, ident_f)
        ident16 = consts.tile([P, P], f16)
        nc.vector.tensor_copy(out=ident16, in_=ident_f)
        ident8 = consts.tile([P, P], f8)
        nc.vector.tensor_copy(out=ident8, in_=ident_f)
        eps_t = consts.tile([P, 1], f32)
        nc.vector.memset(eps_t, EPS)
        mask_t = consts.tile([CHUNK, CHUNK], f32)
        nc.sync.dma_start(out=mask_t, in_=maskT.ap())
        g1 = consts.tile([P, DIM], f32)
        b1 = consts.tile([P, DIM], f32)
        g2 = consts.tile([P, DIM], f32)
        b2 = consts.tile([P, DIM], f32)
        nc.sync.dma_start(out=g1, in_=ln1_g.ap()[None, :].partition_broadcast(P).opt())
        nc.sync.dma_start(out=b1, in_=ln1_b.ap()[None, :].partition_broadcast(P).opt())
        nc.sync.dma_start(out=g2, in_=ln2_g.ap()[None, :].partition_broadcast(P).opt())
        nc.sync.dma_start(out=b2, in_=ln2_b.ap()[None, :].partition_broadcast(P).opt())

        # hT_a: LN2(x2) transposed, fp8; written in phase B, read by FFN level 0
        # and written again by level 1.
        hTa_pool = top.enter_context(tc.tile_pool(name="hTa", bufs=1))
        hT_a = hTa_pool.tile([P, D_T, t_own], f8)

        # ---------------- attention (phases A+B) ----------------
        ab = ExitStack()
        mt_pool = ab.enter_context(tc.tile_pool(name="mt", bufs=1))
        Mt_f = [mt_pool.tile([P, HD], f32, name=f"Mt_f{h}", tag=f"Mt_f{h}") for h in range(HEADS)]
        Mt_s = [mt_pool.tile([P, HD], f16, name=f"Mt_s{h}", tag=f"Mt_s{h}") for h in range(HEADS)]
        for h in range(HEADS):
            nc.vector.memset(Mt_f[h], 0.0)
        ln_w = ab.enter_context(tc.tile_pool(name="ln_w", bufs=1))
        wo_pool = ab.enter_context(tc.tile_pool(name="wo_pool", bufs=1))
        wo_all = wo.ap().rearrange("(kt p) d -> p kt d", p=P)
        wo_ts = []
        for nh in range(2):
            w_t = wo_pool.tile([P, D_T, 512], f16, tag=f"wo{nh}", name=f"wo_full_{nh}")
            nc.scalar.dma_start(out=w_t, in_=wo_all[:, :, nh * 512:(nh + 1) * 512])
            wo_ts.append(w_t)

        # ---------------- Phase A: prefix -> M ----------------
        with ExitStack() as pa:
            a_sb = pa.enter_context(tc.tile_pool(name="A_sb", bufs=1))
            a_ps = pa.enter_context(tc.tile_pool(name="A_ps", bufs=2, space="PSUM"))
            a_w = pa.enter_context(tc.tile_pool(name="A_w", bufs=2))
            for blk in range(n_pre_blk):
                ntt = BLK // P
                hTp = a_sb.tile([P, D_T, BLK], f16, tag="A_hT", bufs=1, name=f"hTp_{blk}")
                for t in range(ntt):
                    x_t = a_sb.tile([P, DIM], f32, tag="A_x", bufs=3, name=f"A_x_{blk}_{t}")
                    nc.sync.dma_start(out=x_t, in_=x_pre.ap()[(blk * ntt + t) * P:(blk * ntt + t + 1) * P, :])
                    h_r = a_sb.tile([P, DIM], f16, tag="A_h", bufs=2, name=f"A_h_{blk}_{t}")
                    _layernorm_tile(nc, ln_w, x_t, g1, b1, eps_t, h_r)
                    for fidx in range(D_T):
                        tps = a_ps.tile([P, P], f16, tag="A_tp", name=f"A_tp_{blk}_{t}_{fidx}")
                        nc.tensor.transpose(tps, h_r[:, fidx * P:(fidx + 1) * P], ident16)
                        nc.scalar.copy(out=hTp[:, fidx, t * P:(t + 1) * P], in_=tps)
                kcp = a_sb.tile([P, ntt, DIM], f16, tag="A_kc", bufs=1, name=f"kcp_{blk}")
                vcp = a_sb.tile([P, ntt, DIM], f16, tag="A_vc", bufs=1, name=f"vcp_{blk}")
                for (w_in, dst) in ((wk, kcp), (wv, vcp)):
                    w_all = w_in.ap().rearrange("(kt p) d -> p kt d", p=P)
                    for nh in range(2):
                        w_t = a_w.tile([P, D_T, 512], f16, tag="A_wt", name=f"A_wt_{blk}_{dst.name}_{nh}")
                        nc.sync.dma_start(out=w_t, in_=w_all[:, :, nh * 512:(nh + 1) * 512])
                        for m in range(ntt):
                            pst = a_ps.tile([P, 512], f32, tag="A_pst", name=f"A_pst_{blk}_{dst.name}_{nh}_{m}")
                            for k in range(D_T):
                                nc.tensor.matmul(pst, hTp[:, k, m * P:(m + 1) * P], w_t[:, k, :],
                                                 start=(k == 0), stop=(k == D_T - 1))
                            nc.scalar.copy(out=dst[:, m, nh * 512:(nh + 1) * 512], in_=pst)
                for h in range(HEADS):
                    pb = (h % 2) * HD
                    mt_ps = a_ps.tile([HD, HD], f32, tag="A_mt", name=f"A_mt_{blk}_{h}")
                    for ch in range(ntt):
                        nc.tensor.matmul(mt_ps, kcp[:, ch, h * HD:(h + 1) * HD],
                                         vcp[:, ch, h * HD:(h + 1) * HD],
                                         start=(ch == 0), stop=(ch == ntt - 1))
                    nc.vector.tensor_add(out=Mt_f[h][pb:pb + HD, :], in0=Mt_f[h][pb:pb + HD, :],
                                         in1=mt_ps)
        for h in range(HEADS):
            pb = (h % 2) * HD
            nc.scalar.copy(out=Mt_s[h][pb:pb + HD, :], in_=Mt_f[h][pb:pb + HD, :])

        # ---------------- Phase B: own tokens ----------------
        with ExitStack() as pbk:
            b_sb = pbk.enter_context(tc.tile_pool(name="B_sb", bufs=1))
            b_ps = pbk.enter_context(tc.tile_pool(name="B_ps", bufs=2, space="PSUM"))
            b_w = pbk.enter_context(tc.tile_pool(name="B_w", bufs=2))
            for blk in range(n_blk):
                ntt = BLK // P
                tok0 = blk * BLK
                hTb = b_sb.tile([P, D_T, BLK], f16, tag="B_hT", bufs=1, name=f"hTb_{blk}")
                for t in range(ntt):
                    x_t = b_sb.tile([P, DIM], f32, tag="B_x", bufs=3, name=f"B_x_{blk}_{t}")
                    nc.sync.dma_start(out=x_t, in_=x_own.ap()[tok0 + t * P:tok0 + (t + 1) * P, :])
                    h_r = b_sb.tile([P, DIM], f16, tag="B_h", bufs=2, name=f"B_h_{blk}_{t}")
                    _layernorm_tile(nc, ln_w, x_t, g1, b1, eps_t, h_r)
                    for fidx in range(D_T):
                        tps = b_ps.tile([P, P], f16, tag="B_tp", name=f"B_tp_{blk}_{t}_{fidx}")
                        nc.tensor.transpose(tps, h_r[:, fidx * P:(fidx + 1) * P], ident16)
                        nc.scalar.copy(out=hTb[:, fidx, t * P:(t + 1) * P], in_=tps)
                # qT, kT feature-major [128, D_T, BLK]
                qT = b_sb.tile([P, D_T, BLK], f16, tag="B_qT", bufs=1, name=f"qT_{blk}")
                kT = b_sb.tile([P, D_T, BLK], f16, tag="B_kT", bufs=1, name=f"kT_{blk}")
                for (w_in, dst) in ((wq, qT), (wk, kT)):
                    w_all = w_in.ap().rearrange("(kt p) d -> p kt d", p=P)
                    for m in range(D_T):
                        w_t = b_w.tile([P, D_T, P], f16, tag="B_wt", name=f"B_wt_{blk}_{dst.name}_{m}")
                        nc.sync.dma_start(out=w_t, in_=w_all[:, :, m * P:(m + 1) * P])
                        pst = b_ps.tile([P, BLK], f32, tag="B_pst", bufs=3, name=f"B_pst_{blk}_{dst.name}_{m}")
                        for k in range(D_T):
                            nc.tensor.matmul(pst, w_t[:, k, :], hTb[:, k, :],
                                             start=(k == 0), stop=(k == D_T - 1))
                        nc.scalar.copy(out=dst[:, m, :], in_=pst)
                # kc, v token-major [128, ntt, DIM]
                kc = b_sb.tile([P, ntt, DIM], f16, tag="B_kc", bufs=1, name=f"kc_{blk}")
                v = b_sb.tile([P, ntt, DIM], f16, tag="B_v", bufs=1, name=f"v_{blk}")
                for (w_in, dst) in ((wk, kc), (wv, v)):
                    w_all = w_in.ap().rearrange("(kt p) d -> p kt d", p=P)
                    for nh in range(2):
                        w_t = b_w.tile([P, D_T, 512], f16, tag="B_wtv", name=f"B_wtv_{blk}_{dst.name}_{nh}")
                        nc.sync.dma_start(out=w_t, in_=w_all[:, :, nh * 512:(nh + 1) * 512])
                        for m in range(ntt):
                            pst = b_ps.tile([P, 512], f32, tag="B_pst", bufs=3, name=f"B_pstv_{blk}_{dst.name}_{nh}_{m}")
                            for k in range(D_T):
                                nc.tensor.matmul(pst, hTb[:, k, m * P:(m + 1) * P], w_t[:, k, :],
                                                 start=(k == 0), stop=(k == D_T - 1))
                            nc.scalar.copy(out=dst[:, m, nh * 512:(nh + 1) * 512], in_=pst)
                # scan: chunk-outer, head-inner; y produced pre-transposed
                yTb = b_sb.tile([P, D_T, BLK], f16, tag="B_yT", bufs=1, name=f"yTb_{blk}")
                for ch in range(ntt):
                    for h in range(HEADS):
                        pb = (h % 2) * HD
                        fi = h // 2
                        qcT = qT[pb:pb + HD, fi, ch * P:(ch + 1) * P]
                        kcT = kT[pb:pb + HD, fi, ch * P:(ch + 1) * P]
                        vc = v[:, ch, h * HD:(h + 1) * HD]
                        kc_s = kc[:, ch, h * HD:(h + 1) * HD]
                        # one PSUM bank per (ch,h): scoresT | yT | Mdelta regions
                        scan_ps = b_ps.tile([P, 512], f32, tag="B_scan", bufs=3,
                                            name=f"scan_{blk}_{ch}_{h}")
                        sc_ps = scan_ps[:, 0:P]
                        yT_ps = scan_ps[0:HD, P:P + P]
                        mt_ps = scan_ps[0:HD, 2 * P:2 * P + HD]
                        # scoresT[e, c] (e = key token, c = query token)
                        nc.tensor.matmul(sc_ps, kcT, qcT, start=True, stop=True)
                        sc_r = b_sb.tile([P, P], f16, tag="B_scr", bufs=3, name=f"scr_{blk}_{ch}_{h}")
                        nc.vector.tensor_mul(out=sc_r, in0=sc_ps, in1=mask_t)
                        # yT[v, c] = intra + inter
                        nc.tensor.matmul(yT_ps, vc, sc_r, start=True, stop=False)
                        nc.tensor.matmul(yT_ps, Mt_s[h][pb:pb + HD, :], qcT, start=False, stop=True)
                        nc.scalar.copy(out=yTb[pb:pb + HD, fi, ch * P:(ch + 1) * P], in_=yT_ps)
                        # M[k, v] += kc^T vc
                        nc.tensor.matmul(mt_ps, kc_s, vc, start=True, stop=True)
                        nc.vector.tensor_add(out=Mt_f[h][pb:pb + HD, :],
                                             in0=Mt_f[h][pb:pb + HD, :], in1=mt_ps)
                        nc.scalar.copy(out=Mt_s[h][pb:pb + HD, :], in_=Mt_f[h][pb:pb + HD, :])
                # attn out + residual, LN2, h2T (fp8)
                for m in range(ntt):
                    ti = (tok0 // P) + m
                    x_t = b_sb.tile([P, DIM], f32, tag="B_x2t", bufs=2, name=f"B_x2t_{blk}_{m}")
                    nc.sync.dma_start(out=x_t, in_=x_own.ap()[tok0 + m * P:tok0 + (m + 1) * P, :])
                    x2_t = b_sb.tile([P, DIM], f32, tag="B_x2", bufs=2, name=f"B_x2_{blk}_{m}")
                    for nh in range(2):
                        pst = b_ps.tile([P, 512], f32, tag="B_pst", bufs=3, name=f"B_at_{blk}_{m}_{nh}")
                        for k in range(D_T):
                            nc.tensor.matmul(pst, yTb[:, k, m * P:(m + 1) * P], wo_ts[nh][:, k, :],
                                             start=(k == 0), stop=(k == D_T - 1))
                        nc.vector.tensor_add(out=x2_t[:, nh * 512:(nh + 1) * 512],
                                             in0=x_t[:, nh * 512:(nh + 1) * 512], in1=pst)
                    nc.scalar.dma_start(out=x2_d[ti], in_=x2_t)
                    h2_r = b_sb.tile([P, DIM], f16, tag="B_h2", bufs=2, name=f"B_h2_{blk}_{m}")
                    _layernorm_tile(nc, ln_w, x2_t, g2, b2, eps_t, h2_r)
                    for fidx in range(D_T):
                        tps = b_ps.tile([P, P], f16, tag="B_tp", name=f"B_h2tp_{blk}_{m}_{fidx}")
                        nc.tensor.transpose(tps, h2_r[:, fidx * P:(fidx + 1) * P], ident16)
                        nc.scalar.copy(out=hT_a[:, fidx, ti * P:(ti + 1) * P], in_=tps)
        ab.close()

        # ---------------- Phase C: CMS FFN, fp8 DoubleRow ----------------
        with ExitStack() as pc:
            ffn = pc.enter_context(tc.tile_pool(name="ffn", bufs=1))
            hT_b = ffn.tile([P, D_T, t_own], f8)
            ug = ffn.tile([P, H_T, t_own], f8)
            w2r = ffn.tile([P, H_T, DIM], f8)   # level-2 w2, resident for token-major down
            w2r_all = w2q.ap()[LEVELS - 1].rearrange("(kt p) d -> p kt d", p=P)
            for kh in range(4):
                HK = H_T // 4
                nc.scalar.dma_start(out=w2r[:, kh * HK:(kh + 1) * HK, :],
                                    in_=w2r_all[:, kh * HK:(kh + 1) * HK, :])
            bias_p = pc.enter_context(tc.tile_pool(name="bias", bufs=1))
            b2_bc = bias_p.tile([P, DIM], f32)
            nc.sync.dma_start(out=b2_bc, in_=cms_b2.ap()[LEVELS - 1][None, :].partition_broadcast(P).opt())
            c_w = pc.enter_context(tc.tile_pool(name="C_w", bufs=3))

            hT_io = [(hT_a, hT_b), (hT_b, hT_a), (hT_a, None)]
            n_tt = t_own // 512
            for lvl in range(LEVELS):
                hT_in, hT_out = hT_io[lvl]
                b1_t = bias_p.tile([P, H_T], f32, tag="b1t", bufs=2, name=f"b1t_{lvl}")
                nc.sync.dma_start(out=b1_t, in_=cms_b1.ap()[lvl].rearrange("(m p) -> p m", p=P))
                if lvl < LEVELS - 1:
                    b2_t = bias_p.tile([P, D_T], f32, tag="b2t", bufs=2, name=f"b2t_{lvl}")
                    nc.sync.dma_start(out=b2_t, in_=cms_b2.ap()[lvl].rearrange("(m p) -> p m", p=P))
                # UP: out ug[hid, tok] = gelu((hT_in^T w1)^T); DoubleRow over k-pairs
                w1_all = w1q.ap()[lvl].rearrange("(kt p) d -> p kt d", p=P)
                with ExitStack() as sub:
                    ps = sub.enter_context(tc.tile_pool(name=f"C{lvl}_up_ps", bufs=2, space="PSUM"))
                    for m in range(H_T):
                        w_t = c_w.tile([P, D_T, P], f8, tag="up_w", name=f"up_w_{lvl}_{m}")
                        nc.sync.dma_start(out=w_t, in_=w1_all[:, :, m * P:(m + 1) * P])
                        psl = [ps.tile([P, 512], f32, tag=f"up_ps{t}", name=f"up_ps_{lvl}_{m}_{t}")
                               for t in range(n_tt)]
                        for kp in range(D_T // 2):
                            for tt in range(n_tt):
                                nc.tensor.matmul(psl[tt], w_t[:, 2 * kp:2 * kp + 2, :],
                                                 hT_in[:, 2 * kp:2 * kp + 2, tt * 512:(tt + 1) * 512],
                                                 start=(kp == 0), stop=(kp == D_T // 2 - 1),
                                                 perf_mode=DR)
                        for tt in range(n_tt):
                            nc.scalar.activation(out=ug[:, m, tt * 512:(tt + 1) * 512], in_=psl[tt],
                                                 func=AF.Gelu_apprx_tanh,
                                                 bias=b1_t[:, m:m + 1], scale=1.0 / W_SCALE)
                if lvl < LEVELS - 1:
                    # DOWN (feature-major): hT_out[d, tok] = ug^T w2 + b2
                    w2_all = w2q.ap()[lvl].rearrange("(kt p) d -> p kt d", p=P)
                    with ExitStack() as sub:
                        ps = sub.enter_context(tc.tile_pool(name=f"C{lvl}_dn_ps", bufs=2, space="PSUM"))
                        for m in range(D_T):
                            w_t = c_w.tile([P, H_T, P], f8, tag="dn_w", name=f"dn_w_{lvl}_{m}")
                            nc.sync.dma_start(out=w_t, in_=w2_all[:, :, m * P:(m + 1) * P])
                            psl = [ps.tile([P, 512], f32, tag=f"dn_ps{t}", name=f"dn_ps_{lvl}_{m}_{t}")
                                   for t in range(n_tt)]
                            for kp in range(H_T // 2):
                                for tt in range(n_tt):
                                    nc.tensor.matmul(psl[tt], w_t[:, 2 * kp:2 * kp + 2, :],
                                                     ug[:, 2 * kp:2 * kp + 2, tt * 512:(tt + 1) * 512],
                                                     start=(kp == 0), stop=(kp == H_T // 2 - 1),
                                                     perf_mode=DR)
                            for tt in range(n_tt):
                                nc.vector.tensor_scalar(out=hT_out[:, m, tt * 512:(tt + 1) * 512],
                                                        in0=psl[tt], scalar1=1.0 / W_SCALE,
                                                        scalar2=b2_t[:, m:m + 1],
                                                        op0=ALU.mult, op1=ALU.add)
                else:
                    # DOWN (token-major) + residual: out = x2 + ug^T-contract w2 + b2
                    with ExitStack() as sub:
                        ps = sub.enter_context(tc.tile_pool(name=f"C{lvl}_dn_ps", bufs=2, space="PSUM"))
                        o_sb = sub.enter_context(tc.tile_pool(name=f"C{lvl}_o", bufs=1))
                        for ti in range(n_own_t):
                            x2_t = o_sb.tile([P, DIM], f32, tag="D_x2", bufs=3, name=f"D_x2_{ti}")
                            nc.sync.dma_start(out=x2_t, in_=x2_d[ti])
                            x2b_t = o_sb.tile([P, DIM], f32, tag="D_x2b", bufs=3, name=f"D_x2b_{ti}")
                            nc.vector.tensor_add(out=x2b_t, in0=x2_t, in1=b2_bc)
                            psl = [ps.tile([P, 512], f32, tag=f"o_ps{nh}", name=f"o_ps_{ti}_{nh}")
                                   for nh in range(2)]
                            for kp in range(H_T // 2):
                                for nh in range(2):
                                    nc.tensor.matmul(psl[nh], ug[:, 2 * kp:2 * kp + 2, ti * P:(ti + 1) * P],
                                                     w2r[:, 2 * kp:2 * kp + 2, nh * 512:(nh + 1) * 512],
                                                     start=(kp == 0), stop=(kp == H_T // 2 - 1),
                                                     perf_mode=DR)
                            o_t = o_sb.tile([P, DIM], f32, tag="D_o", bufs=3, name=f"D_o_{ti}")
                            for nh in range(2):
                                sl = slice(nh * 512, (nh + 1) * 512)
                                nc.scalar.activation(out=o_t[:, sl], in_=psl[nh], func=AF.Copy,
                                                     bias=0.0, scale=1.0 / W_SCALE)
                                nc.vector.tensor_add(out=o_t[:, sl], in0=o_t[:, sl], in1=x2b_t[:, sl])
                            nc.scalar.dma_start(out=out.ap()[ti * P:(ti + 1) * P, :], in_=o_t)

    _split_multi_waits(nc)
    return nc


_NC_CACHE = {}
LAST_RESULT = None


def _get_nc(key, **kw):
    if key not in _NC_CACHE:
        _NC_CACHE[key] = build_kernel(**kw)
    return _NC_CACHE[key]


def _q8(w, scale=W_SCALE):
    return np.clip(np.asarray(w, np.float32) * scale, -240.0, 240.0).astype(ml_dtypes.float8_e4m3)


def kernel(x, ln1_g, ln1_b, wq, wk, wv, wo, ln2_g, ln2_b,
           cms_w1, cms_b1, cms_w2, cms_b2, **extra):
    x = np.asarray(x, np.float32)
    maskT = np.triu(np.ones((CHUNK, CHUNK), np.float32))  # maskT[e,c] = e<=c
    common = {
        "wq": np.asarray(wq, np.float32).astype(np.float16),
        "wk": np.asarray(wk, np.float32).astype(np.float16),
        "wv": np.asarray(wv, np.float32).astype(np.float16),
        "wo": np.asarray(wo, np.float32).astype(np.float16),
        "ln1_g": np.asarray(ln1_g, np.float32), "ln1_b": np.asarray(ln1_b, np.float32),
        "ln2_g": np.asarray(ln2_g, np.float32), "ln2_b": np.asarray(ln2_b, np.float32),
        "w1q": _q8(cms_w1), "cms_b1": np.asarray(cms_b1, np.float32),
        "w2q": _q8(cms_w2), "cms_b2": np.asarray(cms_b2, np.float32),
        "maskT": maskT,
    }
    zeros_pre = np.zeros((T_PRE, DIM), np.float32)
    in_maps = []
    for c in range(N_CORES):
        b, half = c // 2, c % 2
        own = x[b, half * T_OWN:(half + 1) * T_OWN]
        pre = x[b, 0:T_PRE] if half else zeros_pre
        in_maps.append({**common, "x_own": np.ascontiguousarray(own),
                        "x_pre": np.ascontiguousarray(pre)})
    nc = _get_nc("full")
    res = run_bass_kernel_spmd(nc, in_maps, core_ids=list(range(N_CORES)))
    global LAST_RESULT
    LAST_RESULT = res
    out = np.empty((B, S, DIM), np.float32)
    for c in range(N_CORES):
        b, half = c // 2, c % 2
        out[b, half * T_OWN:(half + 1) * T_OWN] = res.results[c]["out"]
    return out
